# revision 8
# baseline (speedup 1.0000x reference)
"""MLA (multi-head latent attention) Bass kernel for 8 TRN2 NeuronCores.

Sharding: 2 batches x 4 head-groups -> 8 cores. Each core computes 8 heads
of one batch end-to-end (q/latent projections, RoPE, causal attention,
o_proj partial). The o_proj RowParallel all-reduce is done on the host
(sum of 4 partials per batch) - no device collectives.

All matmuls run in float32r (TF32: fp32 with 13 low mantissa bits rounded
away) which streams at bf16 rate on the PE for moving dims >= 256 while
keeping ~1e-4 relative accuracy. Inputs are pre-rounded to the f32r grid on
the host so they can be DMA'd directly into f32r tiles.
"""
import sys

sys.path.insert(0, "/opt/trn_rl_repo")

import numpy as np

import bass_rust as _bass_rust
import concourse.bass as bass
import concourse.mybir as mybir
import concourse.tile as tile
from concourse.vector_clock import ScopedClock

# ---------------------------------------------------------------- constants
B, S, D = 2, 2048, 4096
H, DL, DR = 32, 512, 64
HD = D // H  # 128
NCORES = 8
TP = 4  # head groups
HP = H // TP  # 8 heads per core
KC = D // 128  # 32 contraction chunks over D
SC = S // 512  # 4 s-chunks of 512
ST = S // 128  # 16 s-tiles of 128
LC = DL // 128  # 4 latent chunks
QT = (HP * HD) // 128  # 8 qT m-tiles (4 pe + 4 nope after permutation)
SCALE = 1.0 / np.sqrt(np.float32(HD))

F32 = mybir.dt.float32
F32R = mybir.dt.float32r


# ------------------------------------------------------- tile wait legalizer
def _split_waits(nc, insts):
    out = []
    for inst in insts:
        si = getattr(inst, "sync_info", None)
        waits = list(si.on_wait) if (si is not None and si.on_wait) else []
        if len(waits) > 1:
            eng = inst.engine
            for w in waits[:-1]:
                nop = _bass_rust.InstNoOp(
                    name=nc.get_next_instruction_name(), ins=[], outs=[]
                )
                nop.engine = eng
                nop.sync_info = mybir.SyncInfo(on_wait=[w], on_update=[])
                out.append(nop)
            inst.sync_info = mybir.SyncInfo(
                on_wait=[waits[-1]], on_update=list(si.on_update or [])
            )
        out.append(inst)
    return out


class LegalTileContext(tile.TileContext):
    """Walrus here accepts only one sem wait per instruction; split extras
    onto same-engine NoOps placed immediately before the instruction."""

    def _lower_ordered_insts(self, ordered):
        for bb_name in list(ordered.keys()):
            ordered[bb_name][:] = _split_waits(self.nc, ordered[bb_name])
        return super()._lower_ordered_insts(ordered)

    def _drain_and_barrier(self, tick_clock, wait_clock):
        drain_inst = self.nc.sync.drain()
        wait_clock.add_sem_waits(
            drain_inst.ins, ScopedClock({None: tick_clock.global_clock})
        )
        si = getattr(drain_inst.ins, "sync_info", None)
        waits = list(si.on_wait) if (si is not None and si.on_wait) else []
        if len(waits) > 1:
            drain_inst.ins.sync_info = mybir.SyncInfo(
                on_wait=[waits[0]], on_update=list(si.on_update or [])
            )
            for w in waits[1:]:
                d2 = self.nc.sync.drain()
                d2.ins.sync_info = mybir.SyncInfo(on_wait=[w], on_update=[])
        self.nc.all_engine_barrier()
        assert self.sems is not None
        popped = self.nc._tile_sem_poison_stack.pop()
        assert popped is self._sem_poison
        self.nc.clear_and_free_semaphores(list(self.sems.allocated().values()))
        self.nc.all_engine_barrier()


# ------------------------------------------------------------- bass program
def build_bass():
    nc = bass.Bass()
    xT_d = nc.dram_tensor("xT", [128, KC, S], F32R, kind="ExternalInput")
    wq_d = nc.dram_tensor("wq", [128, QT, KC, 128], F32R, kind="ExternalInput")
    wdn_d = nc.dram_tensor("wdn", [128, LC, KC, 128], F32R, kind="ExternalInput")
    wv_d = nc.dram_tensor("wv", [128, LC, HP * HD], F32R, kind="ExternalInput")
    wk_d = nc.dram_tensor("wk", [128, LC, HP * (HD - DR)], F32R, kind="ExternalInput")
    wkpe_d = nc.dram_tensor("wkpe", [128, LC, DR], F32R, kind="ExternalInput")
    wo_d = nc.dram_tensor("wo", [128, HP, D], F32R, kind="ExternalInput")
    cos_d = nc.dram_tensor("cos2", [128, S], F32, kind="ExternalInput")
    sin_d = nc.dram_tensor("sinS", [128, S], F32, kind="ExternalInput")
    mask_d = nc.dram_tensor("masks", [128, 4, 512], F32R, kind="ExternalInput")
    ones_d = nc.dram_tensor("ones", [128, 128], F32R, kind="ExternalInput")
    y_d = nc.dram_tensor("y", [S, D], F32, kind="ExternalOutput")

    with LegalTileContext(nc) as tc:
        with (
            tc.tile_pool(name="const", bufs=1) as constp,
            tc.tile_pool(name="dram", bufs=1, space="DRAM") as dramp,
        ):
            ones_t = constp.tile([128, 128], F32R, tag="ones")
            nc.sync.dma_start(ones_t[:], ones_d[:])

            qT_dram = dramp.tile([128, QT, S], F32R, tag="qTd")
            o8_dram = dramp.tile([HP, HD, S], F32R, tag="o8d")
            kT_dram = dramp.tile([HP, HD, S], F32R, tag="kTd")
            v8_dram = dramp.tile([ST, 128, HP * HD], F32R, tag="v8d")

            with (
                tc.tile_pool(name="rope", bufs=1) as ropep,
                tc.tile_pool(name="lat", bufs=1) as latp,
            ):
                cos_t = ropep.tile([128, S], F32, tag="cos")
                sin_t = ropep.tile([128, S], F32, tag="sin")
                nc.sync.dma_start(cos_t[:], cos_d[:])
                nc.sync.dma_start(sin_t[:], sin_d[:])
                latT = latp.tile([128, LC, S], F32R, tag="latT")

                # ---------------- phase 1: qT (rope'd) + latentT -----------
                with (
                    tc.tile_pool(name="p1x", bufs=3) as xp,
                    tc.tile_pool(name="p1w", bufs=3) as wp,
                    tc.tile_pool(name="p1t", bufs=2) as tp1,
                    tc.tile_pool(name="p1ps", bufs=4, space="PSUM") as psp,
                ):
                    HK = KC // 2  # 16-chunk halves of the contraction
                    for j in range(SC):
                        js = slice(j * 512, (j + 1) * 512)
                        xh = []
                        for half in range(2):
                            xt_ = xp.tile([128, HK, 512], F32R, tag="xh")
                            nc.sync.dma_start(
                                xt_[:], xT_d[:, half * HK : (half + 1) * HK, js]
                            )
                            xh.append(xt_)
                        for t in range(QT + LC):
                            ps = psp.tile([128, 512], F32, tag="ps1")
                            for half in range(2):
                                w = wp.tile([128, HK, 128], F32R, tag="w1")
                                hs = slice(half * HK, (half + 1) * HK)
                                if t < QT:
                                    nc.sync.dma_start(w[:], wq_d[:, t, hs])
                                else:
                                    nc.sync.dma_start(w[:], wdn_d[:, t - QT, hs])
                                for ci in range(HK):
                                    nc.tensor.matmul(
                                        ps[:],
                                        w[:, ci],
                                        xh[half][:, ci],
                                        start=(half == 0 and ci == 0),
                                        stop=(half == 1 and ci == HK - 1),
                                    )
                            if t < 4:
                                # pe q-tile (2 heads x 64 rope dims): RoPE
                                q_sb = tp1.tile([128, 512], F32, tag="qsb")
                                nc.scalar.copy(q_sb[:], ps[:])
                                rot = tp1.tile([128, 512], F32, tag="rot")
                                for half in range(2):
                                    b0 = half * 64
                                    nc.sync.dma_start(
                                        rot[b0 : b0 + 32, :],
                                        q_sb[b0 + 32 : b0 + 64, :],
                                    )
                                    nc.sync.dma_start(
                                        rot[b0 + 32 : b0 + 64, :],
                                        q_sb[b0 : b0 + 32, :],
                                    )
                                t1 = tp1.tile([128, 512], F32, tag="t1")
                                t2 = tp1.tile([128, 512], F32, tag="t2")
                                nc.vector.tensor_mul(t1[:], q_sb[:], cos_t[:, js])
                                nc.vector.tensor_mul(t2[:], rot[:], sin_t[:, js])
                                qf = tp1.tile([128, 512], F32R, tag="qf")
                                nc.vector.tensor_add(qf[:], t1[:], t2[:])
                                nc.sync.dma_start(qT_dram[:, t, js], qf[:])
                            elif t < QT:
                                qf = tp1.tile([128, 512], F32R, tag="qf")
                                nc.scalar.copy(qf[:], ps[:])
                                nc.sync.dma_start(qT_dram[:, t, js], qf[:])
                            else:
                                nc.vector.tensor_copy(latT[:, t - QT, js], ps[:])

                # -------- phase 2: v8, kuT, kpeT (+rope) ------------------
                with (
                    tc.tile_pool(name="kvw", bufs=1) as kvwp,
                    tc.tile_pool(name="kvt", bufs=3) as kvt,
                    tc.tile_pool(name="p2ps", bufs=2, space="PSUM") as psp2,
                ):
                    # v8: out[s:128, d:512], lhsT = latT slice, rhs = wv
                    wv_t = kvwp.tile([128, LC, HP * HD], F32R, tag="wv")
                    nc.sync.dma_start(wv_t[:], wv_d[:])
                    for st in range(ST):
                        for n in range(2):
                            ps = psp2.tile([128, 512], F32, tag="psv")
                            for lc in range(LC):
                                nc.tensor.matmul(
                                    ps[:],
                                    latT[:, lc, st * 128 : (st + 1) * 128],
                                    wv_t[:, lc, n * 512 : (n + 1) * 512],
                                    start=(lc == 0),
                                    stop=(lc == LC - 1),
                                )
                            vt = kvt.tile([128, 512], F32R, tag="vt")
                            nc.vector.tensor_copy(vt[:], ps[:])
                            nc.sync.dma_start(
                                v8_dram[st, :, n * 512 : (n + 1) * 512], vt[:]
                            )

                    # kuT: out[dk:128, s:512]; tile t covers heads 2t, 2t+1
                    wk_t = kvwp.tile([128, LC, HP * (HD - DR)], F32R, tag="wk")
                    nc.sync.dma_start(wk_t[:], wk_d[:])
                    for t in range(4):
                        for j in range(SC):
                            js = slice(j * 512, (j + 1) * 512)
                            ps = psp2.tile([128, 512], F32, tag="psk")
                            for lc in range(LC):
                                nc.tensor.matmul(
                                    ps[:],
                                    wk_t[:, lc, t * 128 : (t + 1) * 128],
                                    latT[:, lc, js],
                                    start=(lc == 0),
                                    stop=(lc == LC - 1),
                                )
                            ku = kvt.tile([128, 512], F32R, tag="ku")
                            nc.vector.tensor_copy(ku[:], ps[:])
                            nc.sync.dma_start(
                                kT_dram[2 * t, DR:HD, js], ku[0:64, :]
                            )
                            nc.sync.dma_start(
                                kT_dram[2 * t + 1, DR:HD, js], ku[64:128, :]
                            )

                    # kpeT [64, S] + rope -> all heads' rows 0:64
                    wkpe_t = kvwp.tile([128, LC, DR], F32R, tag="wkpe")
                    nc.sync.dma_start(wkpe_t[:], wkpe_d[:])
                    for j in range(SC):
                        js = slice(j * 512, (j + 1) * 512)
                        ps = psp2.tile([64, 512], F32, tag="pskp")
                        for lc in range(LC):
                            nc.tensor.matmul(
                                ps[:],
                                wkpe_t[:, lc],
                                latT[:, lc, js],
                                start=(lc == 0),
                                stop=(lc == LC - 1),
                            )
                        kp_sb = kvt.tile([64, 512], F32, tag="kpsb")
                        nc.scalar.copy(kp_sb[:], ps[:])
                        rot = kvt.tile([64, 512], F32, tag="krot")
                        nc.sync.dma_start(rot[0:32, :], kp_sb[32:64, :])
                        nc.sync.dma_start(rot[32:64, :], kp_sb[0:32, :])
                        t1 = kvt.tile([64, 512], F32, tag="kt1")
                        t2 = kvt.tile([64, 512], F32, tag="kt2")
                        nc.vector.tensor_mul(t1[:], kp_sb[:], cos_t[0:64, js])
                        nc.vector.tensor_mul(t2[:], rot[:], sin_t[0:64, js])
                        kpf = kvt.tile([64, 512], F32R, tag="kpf")
                        nc.vector.tensor_add(kpf[:], t1[:], t2[:])
                        for h in range(HP):
                            nc.sync.dma_start(kT_dram[h, 0:DR, js], kpf[:])

            # ---------------- phase 3: attention per head ------------------
            with (
                tc.tile_pool(name="hin", bufs=2) as hinp,
                tc.tile_pool(name="pw", bufs=4) as pwp,
                tc.tile_pool(name="at", bufs=3) as atp,
                tc.tile_pool(name="sps", bufs=3, space="PSUM") as spsp,
                tc.tile_pool(name="aps", bufs=2, space="PSUM") as apsp,
                tc.tile_pool(name="rps", bufs=2, space="PSUM") as rpsp,
                tc.tile_pool(name="amask", bufs=1) as amaskp,
                tc.tile_pool(name="bps", bufs=1, space="PSUM") as bpsp,
            ):
                mask_t = amaskp.tile([128, 4, 512], F32R, tag="mask")
                nc.sync.dma_start(mask_t[:], mask_d[:])
                for h in range(HP):
                    qh = hinp.tile([128, S], F32R, tag="qh")
                    hw = 64 * (h % 2)
                    nc.sync.dma_start(qh[0:64, :], qT_dram[hw : hw + 64, h // 2, :])
                    nc.sync.dma_start(
                        qh[64:128, :], qT_dram[hw : hw + 64, 4 + h // 2, :]
                    )
                    kh = hinp.tile([128, S], F32R, tag="kh")
                    nc.sync.dma_start(kh[:], kT_dram[h])
                    vh = hinp.tile([128, ST, HD], F32R, tag="vh")
                    nc.sync.dma_start(
                        vh[:],
                        v8_dram[:, :, h * HD : (h + 1) * HD].rearrange(
                            "t p d -> p t d"
                        ),
                    )
                    for qb in range(SC):
                        nkb = 4 * qb + 4
                        av_ps = apsp.tile([128, 512], F32, tag="av")
                        r_ps = rpsp.tile([1, 512], F32, tag="r")
                        qs = slice(qb * 512, (qb + 1) * 512)
                        for kb in range(nkb):
                            sc_ps = spsp.tile([128, 512], F32, tag="sc")
                            nc.tensor.matmul(
                                sc_ps[:],
                                kh[:, kb * 128 : (kb + 1) * 128],
                                qh[:, qs],
                                start=True,
                                stop=True,
                            )
                            if kb >= 4 * qb:  # diagonal block: mask after exp
                                praw = pwp.tile([128, 512], F32, tag="praw")
                                nc.scalar.activation(
                                    praw[:],
                                    sc_ps[:],
                                    mybir.ActivationFunctionType.Exp,
                                    scale=float(SCALE),
                                )
                                p_sb = pwp.tile([128, 512], F32R, tag="psb")
                                nc.vector.tensor_mul(
                                    p_sb[:], praw[:], mask_t[:, kb - 4 * qb]
                                )
                            else:
                                p_sb = pwp.tile([128, 512], F32R, tag="psb")
                                nc.scalar.activation(
                                    p_sb[:],
                                    sc_ps[:],
                                    mybir.ActivationFunctionType.Exp,
                                    scale=float(SCALE),
                                )
                            nc.tensor.matmul(
                                av_ps[:],
                                vh[:, kb],
                                p_sb[:],
                                start=(kb == 0),
                                stop=(kb == nkb - 1),
                            )
                            nc.tensor.matmul(
                                r_ps[:],
                                ones_t[:, 0:1],
                                p_sb[:],
                                start=(kb == 0),
                                stop=(kb == nkb - 1),
                            )
                        recip = atp.tile([1, 512], F32R, tag="recip")
                        with nc.allow_low_precision("f32r recip, 1e-4 rel is fine"):
                            nc.vector.reciprocal(recip[:], r_ps[:])
                        bc_ps = bpsp.tile([128, 512], F32, tag="bc")
                        nc.tensor.matmul(
                            bc_ps[:], ones_t[0:1, :], recip[:], start=True, stop=True
                        )
                        bc_sb = atp.tile([128, 512], F32, tag="bcsb")
                        nc.scalar.copy(bc_sb[:], bc_ps[:])
                        o_sb = atp.tile([128, 512], F32R, tag="osb")
                        nc.vector.tensor_mul(o_sb[:], av_ps[:], bc_sb[:])
                        nc.sync.dma_start(o8_dram[h, :, qs], o_sb[:])

            # ------------------- phase 4: o_proj ---------------------------
            with (
                tc.tile_pool(name="ow", bufs=1) as owp,
                tc.tile_pool(name="ox", bufs=2) as oxp,
                tc.tile_pool(name="oy", bufs=3) as oyp,
                tc.tile_pool(name="ops", bufs=4, space="PSUM") as opsp,
            ):
                wo_t = owp.tile([128, HP, D], F32R, tag="wo")
                nc.sync.dma_start(wo_t[:], wo_d[:])
                for st in range(ST):
                    o8j = oxp.tile([128, HP, 128], F32R, tag="o8j")
                    nc.sync.dma_start(
                        o8j[:],
                        o8_dram[:, :, st * 128 : (st + 1) * 128].rearrange(
                            "c p s -> p c s"
                        ),
                    )
                    for n in range(D // 512):
                        ps = opsp.tile([128, 512], F32, tag="psy")
                        for c in range(HP):
                            nc.tensor.matmul(
                                ps[:],
                                o8j[:, c],
                                wo_t[:, c, n * 512 : (n + 1) * 512],
                                start=(c == 0),
                                stop=(c == HP - 1),
                            )
                        y_sb = oyp.tile([128, 512], F32, tag="ysb")
                        nc.scalar.copy(y_sb[:], ps[:])
                        nc.sync.dma_start(
                            y_d[st * 128 : (st + 1) * 128, n * 512 : (n + 1) * 512],
                            y_sb[:],
                        )
    nc.finalize()
    return nc


# ------------------------------------------------------------ host plumbing
def _round_f32r(x):
    x = np.ascontiguousarray(x, dtype=np.float32)
    b = x.view(np.uint32)
    r = ((b.astype(np.uint64) + 0x1000) & 0xFFFFE000).astype(np.uint32)
    return r.view(np.float32)


def _rope_tables():
    inv = 1.0 / (10000.0 ** (np.arange(0, DR, 2, dtype=np.float64) / DR))  # 32
    t = np.arange(S, dtype=np.float64)
    ang = np.outer(inv, t)  # [32, S]
    cos64 = np.cos(np.concatenate([ang, ang], axis=0))  # [64, S]
    sin64 = np.sin(np.concatenate([ang, ang], axis=0))
    sin_signed = np.concatenate([-sin64[0:32], sin64[32:64]], axis=0)
    cos2 = np.concatenate([cos64, cos64], axis=0).astype(np.float32)
    sinS = np.concatenate([sin_signed, sin_signed], axis=0).astype(np.float32)
    return cos2, sinS


def _masks():
    m = np.zeros((128, 4, 512), dtype=np.float32)
    k = np.arange(128)[:, None]
    q = np.arange(512)[None, :]
    for oi, o in enumerate((0, 128, 256, 384)):
        m[:, oi, :] = (k + o <= q).astype(np.float32)
    return m


def prepare_core_inputs(x, Wq, Wdown, Wv, Wk, Wkpe, Wo):
    """Build the 8 per-core input dicts (host sharding + layout + f32r)."""
    cos2, sinS = _rope_tables()
    masks = _round_f32r(_masks())
    ones = _round_f32r(np.ones((128, 128), dtype=np.float32))

    xTs = []
    for b in range(B):
        xt = np.ascontiguousarray(x[b].T)  # [D, S]
        xTs.append(_round_f32r(xt.reshape(KC, 128, S).transpose(1, 0, 2)))

    # wdn[p, lc, c, m] = Wdown[c*128+p, lc*128+m]
    wdn = _round_f32r(Wdown.reshape(KC, 128, LC, 128).transpose(1, 2, 0, 3))

    per_group = {}
    for g in range(TP):
        h0 = g * HP
        cols_pe = np.concatenate(
            [np.arange((h0 + h) * HD, (h0 + h) * HD + DR) for h in range(HP)]
        )
        cols_nope = np.concatenate(
            [np.arange((h0 + h) * HD + DR, (h0 + h + 1) * HD) for h in range(HP)]
        )
        cols = np.concatenate([cols_pe, cols_nope])  # 1024
        wq = _round_f32r(
            Wq[:, cols].reshape(KC, 128, QT, 128).transpose(1, 2, 0, 3)
        )  # [128, QT, KC, 128]
        wv = _round_f32r(
            Wv[:, h0 * HD : (h0 + HP) * HD].reshape(LC, 128, HP * HD).transpose(
                1, 0, 2
            )
        )
        wk = _round_f32r(
            Wk[:, h0 * (HD - DR) : (h0 + HP) * (HD - DR)]
            .reshape(LC, 128, HP * (HD - DR))
            .transpose(1, 0, 2)
        )
        wkpe = _round_f32r(Wkpe.reshape(LC, 128, DR).transpose(1, 0, 2))
        wo = _round_f32r(
            Wo[h0 * HD : (h0 + HP) * HD, :].reshape(HP, 128, D).transpose(1, 0, 2)
        )
        per_group[g] = dict(wq=wq, wv=wv, wk=wk, wkpe=wkpe, wo=wo)

    in_maps = []
    for core in range(NCORES):
        b = core // TP
        g = core % TP
        pg = per_group[g]
        in_maps.append(
            {
                "xT": xTs[b],
                "wq": pg["wq"],
                "wdn": wdn,
                "wv": pg["wv"],
                "wk": pg["wk"],
                "wkpe": pg["wkpe"],
                "wo": pg["wo"],
                "cos2": cos2,
                "sinS": sinS,
                "masks": masks,
                "ones": ones,
            }
        )
    return in_maps


_NC_CACHE = {}


def get_nc():
    if "nc" not in _NC_CACHE:
        _NC_CACHE["nc"] = build_bass()
    return _NC_CACHE["nc"]


def kernel(x, Wq, Wdown, Wv, Wk, Wkpe, Wo, mask=None):
    from concourse.bass_utils import run_bass_kernel_spmd

    in_maps = prepare_core_inputs(
        np.asarray(x, np.float32),
        np.asarray(Wq, np.float32),
        np.asarray(Wdown, np.float32),
        np.asarray(Wv, np.float32),
        np.asarray(Wk, np.float32),
        np.asarray(Wkpe, np.float32),
        np.asarray(Wo, np.float32),
    )
    nc = get_nc()
    res = run_bass_kernel_spmd(nc, in_maps, core_ids=list(range(NCORES)))
    out = np.zeros((B, S, D), dtype=np.float32)
    for core in range(NCORES):
        out[core // TP] += res.results[core]["y"]
    return out
